# revision 7
# baseline (speedup 1.0000x reference)
"""EpisodicMemory (binary merge-tree build + 7-slot attention read) on 8
Trainium2 NeuronCores via Bass/Tile.

Algebraic reshaping vs the reference:
  cat([hL, hR, hL, hR]) @ W.T  ==  [hL, hR] @ Wcat.T
  with Wcat = [W1+W3 | W2+W4]  (W = [W1 W2 W3 W4] column blocks of [3d, d]),
  and the [n/2, 2d] "pair matrix" is just the level's node matrix reshaped.
This halves the GEMM FLOPs and turns every tree level into one plain GEMM.

On-chip layout is feature-major: a level's nodes live in SBUF as
[128 partitions, 8 k-tiles, n_nodes] fp16; Wcat.T is the stationary matmul
operand resident in SBUF ([128, 16, 3072] fp16); level pre-activations are
built in PSUM (fp32) as 24 f-tiles x n_nodes; gates run on Scalar/Vector
engines in fp32; the next level is written back as fp16.

Sharding: data-parallel over contiguous 1024-node blocks of the queue (each
core reduces its own subtree, 10 levels); the 8 per-core roots (4 KB fp32)
are exchanged via direct SBUF->SBUF remote-DMA broadcast (receive slot
selected by an 8-way switch on the partition id — much faster than the
ncfw AllGather, whose entry/step latency costs ~80us here); top 3 levels
(7 nodes) are recomputed on every core; attention read is data-parallel
over the 64-row batch (8 rows per core). A tiny dummy collective keeps the
NEFF cc_enabled so the runtime gang-schedules all 8 ranks (without it,
rank starts skew by milliseconds and the root exchange stalls).

fp16 matmul operands keep the tensor engine at full rate at every level size
(fp32 would be 4-pass, fp32r has a 4x penalty below 256 moving columns) and
give ~4e-4 relative error end-to-end (fp32 PSUM accumulation, fp32 gates).
"""

from contextlib import ExitStack

import numpy as np

D = 1024
N_NODES = 8192
BATCH = 64
N_CORES = 8
PER_CORE = N_NODES // N_CORES
BPC = BATCH // N_CORES
FT = 24   # 3072 / 128 output feature tiles
KT = 16   # 2048 / 128 contraction tiles
G = 8     # 1024 / 128 feature tiles of one gate
N_WARMUP_MM = 110  # HAM warmup matmuls issued while input DMAs land

_CACHE = {}


def _build_nc(has_bias):
    import concourse.mybir as mybir
    import concourse.tile as tile
    from concourse import bacc
    from concourse.masks import make_identity

    dt = mybir.dt
    f32, f16 = dt.float32, dt.float16
    AF = mybir.ActivationFunctionType
    OP = mybir.AluOpType
    big_min_pairs = 128

    nc = bacc.Bacc("TRN2", target_bir_lowering=False, debug=False,
                   num_devices=N_CORES)

    # wt host layout: [24 fb, 128 p, 16 t, 128 f2]  (contiguous DMA chunks)
    wt_d = nc.dram_tensor("wt", [FT, 128, KT, 128], f16, kind="ExternalInput").ap()
    qt_d = nc.dram_tensor("qt", [G, 128, PER_CORE], f16, kind="ExternalInput").ap()
    ct_d = nc.dram_tensor("ctt", [D, BPC], f32, kind="ExternalInput").ap()
    b_d = nc.dram_tensor("bb", [FT, 128], f32, kind="ExternalInput").ap()
    out_d = nc.dram_tensor("out", [BPC, D], f32, kind="ExternalOutput").ap()

    core_rounds = []
    p = PER_CORE // 2
    while p >= 1:
        core_rounds.append(p)
        p //= 2
    top_rounds = [4, 2, 1]
    all_rounds = core_rounds + top_rounds
    mem_slot = {1: (0, 1), 2: (1, 3), 4: (3, 7)}
    n_mem_rounds = 3

    with tile.TileContext(nc) as tc:
        with ExitStack() as ctx:
            const = ctx.enter_context(tc.tile_pool(name="const", bufs=1))
            lvp = ctx.enter_context(tc.tile_pool(name="lvp", bufs=2))
            gate = ctx.enter_context(tc.tile_pool(name="gate", bufs=3))
            dram = ctx.enter_context(tc.tile_pool(name="dram", bufs=1, space="DRAM"))
            psum_ctx = ExitStack()
            psum = psum_ctx.enter_context(
                tc.tile_pool(name="psumt", bufs=2, space="PSUM"))

            # dummy collective: forces cc_enabled so the runtime gang-
            # schedules all 8 ranks (aligned starts); runs during the DMA
            # prologue, off the critical path
            dumb_in = dram.tile([1, 8], f32)
            dumb_out = dram.tile([N_CORES, 8], f32, addr_space="Shared")
            zt = gate.tile([1, 8], f32, tag="zt", name="zt")
            nc.vector.memset(zt, 0.0)
            nc.sync.dma_start(dumb_in[:], zt)
            nc.gpsimd.collective_compute(
                "AllGather", mybir.AluOpType.bypass,
                ins=[dumb_in.opt()], outs=[dumb_out.opt()],
                replica_groups=[list(range(N_CORES))],
            )

            # PE warmup (HAM un-throttle) while input DMAs land
            ident = const.tile([128, 128], f32)
            make_identity(nc, ident)
            wu16 = const.tile([128, 128], f16)
            nc.vector.tensor_copy(wu16, ident)
            wu_ps = psum.tile([128, 128], f32, tag="wu", bufs=1, name="wu_ps")
            for _ in range(N_WARMUP_MM):
                nc.tensor.matmul(wu_ps, wu16, wu16, start=True, stop=True)

            qt_sb = const.tile([128, G, PER_CORE], f16)
            for t in range(G):
                nc.sync.dma_start(qt_sb[:, t, :], qt_d[t])

            wt_sb = const.tile([128, KT, 3072], f16)
            for g in range(G):  # f-blocks ordered by first use
                for fb in (g, g + G, g + 2 * G):
                    # split f-blocks across DMA queues (one engine's ~31GB/s
                    # would gate the first matmul group); finest split for
                    # the first group
                    nh = 4 if g == 0 else 2
                    kq = KT // nh
                    for h in range(nh):
                        nc.sync.dma_start(
                            wt_sb[:, kq * h:kq * (h + 1),
                                  128 * fb:128 * (fb + 1)],
                            wt_d[fb][:, kq * h:kq * (h + 1), :])

            b_sb = const.tile([128, FT], f32)
            nc.sync.dma_start(b_sb[:], b_d.rearrange("j p -> p j"))

            ct_sb = const.tile([128, G, BPC], f32)
            nc.sync.dma_start(ct_sb[:], ct_d.rearrange("(t p) b -> p t b", p=128))

            mem_fm = const.tile([128, G, 7], f32)  # MEM, feature-major

            def rhs_for(cur, t):
                if t < G:
                    return cur[:, t, 0::2]
                return cur[:, t - G, 1::2]

            def act(out, in_, fn, fcol):
                if has_bias:
                    nc.scalar.activation(out, in_, fn,
                                         bias=b_sb[:, fcol:fcol + 1])
                else:
                    nc.scalar.activation(out, in_, fn)

            def gate_chain(nxt_view, mem_dst, ig, fg, toc, soc, hL, hR, shape):
                dd = gate.tile(list(shape), f32, tag="dd", name="dd")
                uu = gate.tile(list(shape), f32, tag="uu", name="uu")
                tu = gate.tile(list(shape), f32, tag="tu", name="tu")
                nc.vector.tensor_sub(dd, hL, hR)
                nc.vector.tensor_tensor(dd, fg, dd, OP.mult)
                nc.vector.tensor_add(dd, dd, hR)
                nc.vector.tensor_tensor(uu, ig, toc, OP.mult)
                nc.vector.tensor_add(uu, uu, dd)
                nc.scalar.activation(tu, uu, AF.Tanh)
                nc.vector.tensor_tensor(nxt_view, tu, soc, OP.mult)
                if mem_dst is not None:
                    nc.vector.tensor_tensor(mem_dst, tu, soc, OP.mult)

            def round_big(cur, nxt, P, mem_dst):
                for g in range(G):
                    ps = psum.tile([128, 3, P], f32, tag="ps", name="ps")
                    for c in range(3):
                        for t in range(KT):
                            nc.tensor.matmul(
                                ps[:, c, :],
                                wt_sb[:, t, 128 * (g + 8 * c):128 * (g + 8 * c + 1)],
                                rhs_for(cur, t),
                                start=(t == 0), stop=(t == KT - 1))
                    ig = gate.tile([128, P], f32, tag="ig", name="ig")
                    fg = gate.tile([128, P], f32, tag="fg", name="fg")
                    toc = gate.tile([128, P], f32, tag="toc", name="toc")
                    soc = gate.tile([128, P], f32, tag="soc", name="soc")
                    act(ig, ps[:, 0, :], AF.Sigmoid, g)
                    act(fg, ps[:, 1, :], AF.Sigmoid, g + 8)
                    act(toc, ps[:, 2, :], AF.Tanh, g + 16)
                    act(soc, ps[:, 2, :], AF.Sigmoid, g + 16)
                    md = mem_dst[:, g, :] if mem_dst is not None else None
                    gate_chain(nxt[:, g, :], md, ig, fg, toc, soc,
                               cur[:, g, 0::2], cur[:, g, 1::2], (128, P))

            def round_small(cur, nxt, P, mem_dst):
                ps = psum.tile([128, FT, P], f32, tag="ps", name="ps")
                for gi in range(G):
                    for c in range(3):
                        j = gi + 8 * c
                        for t in range(KT):
                            nc.tensor.matmul(
                                ps[:, j, :], wt_sb[:, t, 128 * j:128 * (j + 1)],
                                rhs_for(cur, t),
                                start=(t == 0), stop=(t == KT - 1))
                for ch in range(2):  # gates in 2 chunks for pipelining
                    gs = slice(4 * ch, 4 * (ch + 1))
                    shape = (128, 4, P)
                    ig = gate.tile(list(shape), f32, tag="ig", name="ig")
                    fg = gate.tile(list(shape), f32, tag="fg", name="fg")
                    toc = gate.tile(list(shape), f32, tag="toc", name="toc")
                    soc = gate.tile(list(shape), f32, tag="soc", name="soc")
                    if has_bias:
                        for k in range(4):
                            g = 4 * ch + k
                            act(ig[:, k, :], ps[:, g, :], AF.Sigmoid, g)
                            act(fg[:, k, :], ps[:, g + 8, :], AF.Sigmoid, g + 8)
                            act(toc[:, k, :], ps[:, g + 16, :], AF.Tanh, g + 16)
                            act(soc[:, k, :], ps[:, g + 16, :], AF.Sigmoid,
                                g + 16)
                    else:
                        nc.scalar.activation(
                            ig, ps[:, gs, :], AF.Sigmoid)
                        nc.scalar.activation(
                            fg, ps[:, slice(gs.start + 8, gs.stop + 8), :],
                            AF.Sigmoid)
                        nc.scalar.activation(
                            toc, ps[:, slice(gs.start + 16, gs.stop + 16), :],
                            AF.Tanh)
                        nc.scalar.activation(
                            soc, ps[:, slice(gs.start + 16, gs.stop + 16), :],
                            AF.Sigmoid)
                    md = mem_dst[:, gs, :] if mem_dst is not None else None
                    gate_chain(nxt[:, gs, :], md, ig, fg, toc, soc,
                               cur[:, gs, 0::2], cur[:, gs, 1::2], shape)

            def run_round(cur, P, collect_mem):
                nxt = lvp.tile([128, G, P], f16, tag="lv", name="lv")
                mem_dst = None
                if collect_mem and P in mem_slot:
                    lo, hi = mem_slot[P]
                    mem_dst = mem_fm[:, :, lo:hi]
                if P >= big_min_pairs:
                    round_big(cur, nxt, P, mem_dst)
                else:
                    round_small(cur, nxt, P, mem_dst)
                return nxt

            cur = qt_sb
            for i, P in enumerate(core_rounds):
                cur = run_round(cur, P, len(all_rounds) - i <= n_mem_rounds)

            # all-gather the 8 per-core roots: direct SBUF->SBUF remote-DMA
            # push of the 4KB fp32 root to all peers; receive slot = own rank
            # via an 8-way switch on the partition id
            root32 = const.tile([128, G, 1], f32)
            nc.vector.tensor_copy(root32, cur)
            nxt16 = lvp.tile([128, G, N_CORES], f16, tag="lv", name="lv")
            roots_nm = const.tile([128, N_CORES, G], f32)
            recv_sem = nc.alloc_semaphore("rdma_recv")
            send_sem = nc.alloc_semaphore("rdma_send")
            prep_sem = nc.alloc_semaphore("rdma_prep")
            gp = nc.gpsimd
            with tc.tile_critical():
                pid = gp.partition_id()
                for case in gp.Switch(pid, N_CORES):
                    gp.remote_dma_broadcast(
                        roots_nm[:, case, :], root32[:, :, 0],
                        remote_sem=recv_sem, local_sem=send_sem,
                        rdests=[(0, k) for k in range(N_CORES)],
                    ).then_inc(prep_sem, 1)
                gp.wait_ge(prep_sem, 1)
                gp.trigger_dma(1)
                cpy = nc.vector.tensor_copy(
                    nxt16, roots_nm.rearrange("p n t -> p t n"))
                cpy._wait_ge(recv_sem, 2 * N_CORES)
            cur = nxt16

            for P in top_rounds:
                cur = run_round(cur, P, True)

            # attention read over this core's batch slice
            psum_ctx.close()
            psa = ctx.enter_context(
                tc.tile_pool(name="psuma", bufs=1, space="PSUM"))
            att_ps = psa.tile([BPC, 7], f32, tag="att", name="att_ps")
            for t in range(G):
                nc.tensor.matmul(att_ps, ct_sb[:, t, :], mem_fm[:, t, :],
                                 start=(t == 0), stop=(t == G - 1))
            negmax = gate.tile([BPC, 1], f32, tag="negmax", name="negmax")
            nc.vector.tensor_reduce(negmax, att_ps, mybir.AxisListType.X,
                                    OP.max, negate=True)
            expv = gate.tile([BPC, 7], f32, tag="expv", name="expv")
            nc.scalar.activation(expv, att_ps, AF.Exp, bias=negmax)
            ssum = gate.tile([BPC, 1], f32, tag="ssum", name="ssum")
            nc.vector.tensor_reduce(ssum, expv, mybir.AxisListType.X, OP.add)
            rinv = gate.tile([BPC, 1], f32, tag="rinv", name="rinv")
            nc.vector.reciprocal(rinv, ssum)
            alpha = gate.tile([BPC, 7], f32, tag="alpha", name="alpha")
            nc.vector.tensor_scalar_mul(alpha, expv, rinv)

            memr = gate.tile([7, G, 128], f32, tag="memr", name="memr")
            for t in range(G):
                tps = psa.tile([7, 128], f32, tag="tps", name="tps")
                nc.tensor.transpose(tps, mem_fm[:, t, :], ident)
                nc.vector.tensor_copy(memr[:, t, :], tps)
            atp = psa.tile([7, BPC], f32, tag="atp", name="atp")
            nc.tensor.transpose(atp, alpha, ident[0:BPC, 0:BPC])
            alphat = gate.tile([7, BPC], f32, tag="alphat", name="alphat")
            nc.vector.tensor_copy(alphat, atp)

            out_sb = gate.tile([BPC, D], f32, tag="out_sb", name="out_sb")
            for half in range(2):
                ops = psa.tile([BPC, 512], f32, tag="ops", name="ops")
                nc.tensor.matmul(ops, alphat, memr[:, 4 * half:4 * (half + 1), :])
                nc.vector.tensor_copy(out_sb[:, 512 * half:512 * (half + 1)], ops)
            nc.sync.dma_start(out_d[:], out_sb[:])

    nc.compile()
    return nc


def _prep_inputs(queue, c_t, W, b):
    queue = np.asarray(queue, dtype=np.float32)
    c_t = np.asarray(c_t, dtype=np.float32)
    W = np.asarray(W, dtype=np.float32)
    b = np.asarray(b, dtype=np.float32)
    d = D
    WL = W[:, :d] + W[:, 2 * d:3 * d]
    WR = W[:, d:2 * d] + W[:, 3 * d:]
    wcat_t = np.concatenate([WL, WR], axis=1).T  # [2048, 3072]
    wt = np.ascontiguousarray(
        wcat_t.reshape(KT, 128, FT, 128).transpose(2, 1, 0, 3)).astype(np.float16)
    bb = np.ascontiguousarray(b.reshape(FT, 128)).astype(np.float32)
    in_maps = []
    for i in range(N_CORES):
        qs = queue[i * PER_CORE:(i + 1) * PER_CORE]
        qt = np.ascontiguousarray(
            qs.T.reshape(G, 128, PER_CORE)).astype(np.float16)
        ctt = np.ascontiguousarray(c_t[i * BPC:(i + 1) * BPC].T).astype(np.float32)
        in_maps.append({"wt": wt, "qt": qt, "ctt": ctt, "bb": bb})
    return in_maps


def kernel(queue, c_t, W, b):
    has_bias = bool(np.any(np.asarray(b)))
    nc = _CACHE.get(has_bias)
    if nc is None:
        nc = _build_nc(has_bias)
        _CACHE[has_bias] = nc
    in_maps = _prep_inputs(queue, c_t, W, b)
    from concourse.bass_utils import run_bass_kernel_spmd
    res = run_bass_kernel_spmd(nc, in_maps, core_ids=list(range(N_CORES)))
    return np.concatenate([r["out"] for r in res.results], axis=0)


# revision 8
# speedup vs baseline: 1.0289x; 1.0289x over previous
"""EpisodicMemory (binary merge-tree build + 7-slot attention read) on 8
Trainium2 NeuronCores via Bass/Tile.

Algebraic reshaping vs the reference:
  cat([hL, hR, hL, hR]) @ W.T  ==  [hL, hR] @ Wcat.T
  with Wcat = [W1+W3 | W2+W4]  (W = [W1 W2 W3 W4] column blocks of [3d, d]),
  and the [n/2, 2d] "pair matrix" is just the level's node matrix reshaped.
This halves the GEMM FLOPs and turns every tree level into one plain GEMM.

On-chip layout is feature-major: a level's nodes live in SBUF as
[128 partitions, 8 k-tiles, n_nodes] fp16; Wcat.T is the stationary matmul
operand resident in SBUF ([128, 16, 3072] fp16); level pre-activations are
built in PSUM (fp32) as 24 f-tiles x n_nodes; gates run on Scalar/Vector
engines in fp32; the next level is written back as fp16.

Sharding: data-parallel over contiguous 1024-node blocks of the queue (each
core reduces its own subtree, 10 levels); the 8 per-core roots (4 KB fp32)
are exchanged via direct SBUF->SBUF remote-DMA broadcast (receive slot
selected by an 8-way switch on the partition id — much faster than the
ncfw AllGather, whose entry/step latency costs ~80us here); top 3 levels
(7 nodes) are recomputed on every core; attention read is data-parallel
over the 64-row batch (8 rows per core). A tiny dummy collective keeps the
NEFF cc_enabled so the runtime gang-schedules all 8 ranks (without it,
rank starts skew by milliseconds and the root exchange stalls).

fp16 matmul operands keep the tensor engine at full rate at every level size
(fp32 would be 4-pass, fp32r has a 4x penalty below 256 moving columns) and
give ~4e-4 relative error end-to-end (fp32 PSUM accumulation, fp32 gates).
"""

from contextlib import ExitStack

import numpy as np

D = 1024
N_NODES = 8192
BATCH = 64
N_CORES = 8
PER_CORE = N_NODES // N_CORES
BPC = BATCH // N_CORES
FT = 24   # 3072 / 128 output feature tiles
KT = 16   # 2048 / 128 contraction tiles
G = 8     # 1024 / 128 feature tiles of one gate
N_WARMUP_MM = 135  # HAM warmup matmuls issued while input DMAs land

_CACHE = {}


def _build_nc(has_bias):
    import concourse.mybir as mybir
    import concourse.tile as tile
    from concourse import bacc
    from concourse.masks import make_identity

    dt = mybir.dt
    f32, f16 = dt.float32, dt.float16
    AF = mybir.ActivationFunctionType
    OP = mybir.AluOpType
    big_min_pairs = 128

    nc = bacc.Bacc("TRN2", target_bir_lowering=False, debug=False,
                   num_devices=N_CORES)

    # wt host layout: [24 fb, 128 p, 16 t, 128 f2]  (contiguous DMA chunks)
    wt_d = nc.dram_tensor("wt", [FT, 128, KT, 128], f16, kind="ExternalInput").ap()
    qt_d = nc.dram_tensor("qt", [G, 128, PER_CORE], f16, kind="ExternalInput").ap()
    ct_d = nc.dram_tensor("ctt", [D, BPC], f32, kind="ExternalInput").ap()
    b_d = nc.dram_tensor("bb", [FT, 128], f32, kind="ExternalInput").ap()
    out_d = nc.dram_tensor("out", [BPC, D], f32, kind="ExternalOutput").ap()

    core_rounds = []
    p = PER_CORE // 2
    while p >= 1:
        core_rounds.append(p)
        p //= 2
    top_rounds = [4, 2, 1]
    all_rounds = core_rounds + top_rounds
    mem_slot = {1: (0, 1), 2: (1, 3), 4: (3, 7)}
    n_mem_rounds = 3

    with tile.TileContext(nc) as tc:
        with ExitStack() as ctx:
            const = ctx.enter_context(tc.tile_pool(name="const", bufs=1))
            lvp = ctx.enter_context(tc.tile_pool(name="lvp", bufs=2))
            gate = ctx.enter_context(tc.tile_pool(name="gate", bufs=3))
            dram = ctx.enter_context(tc.tile_pool(name="dram", bufs=1, space="DRAM"))
            psum_ctx = ExitStack()
            psum = psum_ctx.enter_context(
                tc.tile_pool(name="psumt", bufs=2, space="PSUM"))

            # dummy collective: forces cc_enabled so the runtime gang-
            # schedules all 8 ranks (aligned starts); runs during the DMA
            # prologue, off the critical path
            dumb_in = dram.tile([1, 8], f32)
            dumb_out = dram.tile([N_CORES, 8], f32, addr_space="Shared")
            zt = gate.tile([1, 8], f32, tag="zt", name="zt")
            nc.vector.memset(zt, 0.0)
            nc.sync.dma_start(dumb_in[:], zt)
            nc.gpsimd.collective_compute(
                "AllGather", mybir.AluOpType.bypass,
                ins=[dumb_in.opt()], outs=[dumb_out.opt()],
                replica_groups=[list(range(N_CORES))],
            )

            # PE warmup (HAM un-throttle) while input DMAs land
            ident = const.tile([128, 128], f32)
            make_identity(nc, ident)
            wu16 = const.tile([128, 128], f16)
            nc.vector.tensor_copy(wu16, ident)
            wu_ps = psum.tile([128, 128], f32, tag="wu", bufs=1, name="wu_ps")
            for _ in range(N_WARMUP_MM):
                nc.tensor.matmul(wu_ps, wu16, wu16, start=True, stop=True)

            qt_sb = const.tile([128, G, PER_CORE], f16)
            for t in range(G):
                nc.sync.dma_start(qt_sb[:, t, :], qt_d[t])

            wt_sb = const.tile([128, KT, 3072], f16)
            for g in range(G):  # f-blocks ordered by first use
                for fb in (g, g + G, g + 2 * G):
                    # split f-blocks across DMA queues (one engine's ~31GB/s
                    # would gate the first matmul group); finest split for
                    # the first group
                    nh = 4 if g == 0 else 2
                    kq = KT // nh
                    for h in range(nh):
                        nc.sync.dma_start(
                            wt_sb[:, kq * h:kq * (h + 1),
                                  128 * fb:128 * (fb + 1)],
                            wt_d[fb][:, kq * h:kq * (h + 1), :])

            b_sb = const.tile([128, FT], f32)
            nc.sync.dma_start(b_sb[:], b_d.rearrange("j p -> p j"))

            ct_sb = const.tile([128, G, BPC], f32)
            nc.sync.dma_start(ct_sb[:], ct_d.rearrange("(t p) b -> p t b", p=128))

            mem_fm = const.tile([128, G, 7], f32)  # MEM, feature-major

            def rhs_for(cur, t):
                if t < G:
                    return cur[:, t, 0::2]
                return cur[:, t - G, 1::2]

            def act(out, in_, fn, fcol):
                if has_bias:
                    nc.scalar.activation(out, in_, fn,
                                         bias=b_sb[:, fcol:fcol + 1])
                else:
                    nc.scalar.activation(out, in_, fn)

            def gate_chain(nxt_view, mem_dst, ig, fg, toc, soc, hL, hR, shape):
                dd = gate.tile(list(shape), f32, tag="dd", name="dd")
                uu = gate.tile(list(shape), f32, tag="uu", name="uu")
                tu = gate.tile(list(shape), f32, tag="tu", name="tu")
                nc.vector.tensor_sub(dd, hL, hR)
                nc.vector.tensor_tensor(dd, fg, dd, OP.mult)
                nc.vector.tensor_add(dd, dd, hR)
                nc.vector.tensor_tensor(uu, ig, toc, OP.mult)
                nc.vector.tensor_add(uu, uu, dd)
                nc.scalar.activation(tu, uu, AF.Tanh)
                nc.vector.tensor_tensor(nxt_view, tu, soc, OP.mult)
                if mem_dst is not None:
                    nc.vector.tensor_tensor(mem_dst, tu, soc, OP.mult)

            def round_big(cur, nxt, P, mem_dst):
                for g in range(G):
                    ps = psum.tile([128, 3, P], f32, tag="ps", name="ps")
                    for c in range(3):
                        for t in range(KT):
                            nc.tensor.matmul(
                                ps[:, c, :],
                                wt_sb[:, t, 128 * (g + 8 * c):128 * (g + 8 * c + 1)],
                                rhs_for(cur, t),
                                start=(t == 0), stop=(t == KT - 1))
                    ig = gate.tile([128, P], f32, tag="ig", name="ig")
                    fg = gate.tile([128, P], f32, tag="fg", name="fg")
                    toc = gate.tile([128, P], f32, tag="toc", name="toc")
                    soc = gate.tile([128, P], f32, tag="soc", name="soc")
                    act(ig, ps[:, 0, :], AF.Sigmoid, g)
                    act(fg, ps[:, 1, :], AF.Sigmoid, g + 8)
                    act(toc, ps[:, 2, :], AF.Tanh, g + 16)
                    act(soc, ps[:, 2, :], AF.Sigmoid, g + 16)
                    md = mem_dst[:, g, :] if mem_dst is not None else None
                    gate_chain(nxt[:, g, :], md, ig, fg, toc, soc,
                               cur[:, g, 0::2], cur[:, g, 1::2], (128, P))

            def round_small(cur, nxt, P, mem_dst):
                ps = psum.tile([128, FT, P], f32, tag="ps", name="ps")
                for gi in range(G):
                    for c in range(3):
                        j = gi + 8 * c
                        for t in range(KT):
                            nc.tensor.matmul(
                                ps[:, j, :], wt_sb[:, t, 128 * j:128 * (j + 1)],
                                rhs_for(cur, t),
                                start=(t == 0), stop=(t == KT - 1))
                for ch in range(2):  # gates in 2 chunks for pipelining
                    gs = slice(4 * ch, 4 * (ch + 1))
                    shape = (128, 4, P)
                    ig = gate.tile(list(shape), f32, tag="ig", name="ig")
                    fg = gate.tile(list(shape), f32, tag="fg", name="fg")
                    toc = gate.tile(list(shape), f32, tag="toc", name="toc")
                    soc = gate.tile(list(shape), f32, tag="soc", name="soc")
                    if has_bias:
                        for k in range(4):
                            g = 4 * ch + k
                            act(ig[:, k, :], ps[:, g, :], AF.Sigmoid, g)
                            act(fg[:, k, :], ps[:, g + 8, :], AF.Sigmoid, g + 8)
                            act(toc[:, k, :], ps[:, g + 16, :], AF.Tanh, g + 16)
                            act(soc[:, k, :], ps[:, g + 16, :], AF.Sigmoid,
                                g + 16)
                    else:
                        nc.scalar.activation(
                            ig, ps[:, gs, :], AF.Sigmoid)
                        nc.scalar.activation(
                            fg, ps[:, slice(gs.start + 8, gs.stop + 8), :],
                            AF.Sigmoid)
                        nc.scalar.activation(
                            toc, ps[:, slice(gs.start + 16, gs.stop + 16), :],
                            AF.Tanh)
                        nc.scalar.activation(
                            soc, ps[:, slice(gs.start + 16, gs.stop + 16), :],
                            AF.Sigmoid)
                    md = mem_dst[:, gs, :] if mem_dst is not None else None
                    gate_chain(nxt[:, gs, :], md, ig, fg, toc, soc,
                               cur[:, gs, 0::2], cur[:, gs, 1::2], shape)

            def run_round(cur, P, collect_mem):
                nxt = lvp.tile([128, G, P], f16, tag="lv", name="lv")
                mem_dst = None
                if collect_mem and P in mem_slot:
                    lo, hi = mem_slot[P]
                    mem_dst = mem_fm[:, :, lo:hi]
                if P >= big_min_pairs:
                    round_big(cur, nxt, P, mem_dst)
                else:
                    round_small(cur, nxt, P, mem_dst)
                return nxt

            cur = qt_sb
            for i, P in enumerate(core_rounds):
                cur = run_round(cur, P, len(all_rounds) - i <= n_mem_rounds)

            # all-gather the 8 per-core roots: direct SBUF->SBUF remote-DMA
            # push of the 4KB fp32 root to all peers; receive slot = own rank
            # via an 8-way switch on the partition id
            root16 = const.tile([128, G, 1], f16)
            nc.vector.tensor_copy(root16, cur)
            nxt16 = lvp.tile([128, G, N_CORES], f16, tag="lv", name="lv")
            roots_nm = const.tile([128, N_CORES, G], f16)
            recv_sem = nc.alloc_semaphore("rdma_recv")
            send_sem = nc.alloc_semaphore("rdma_send")
            prep_sem = nc.alloc_semaphore("rdma_prep")
            gp = nc.gpsimd
            with tc.tile_critical():
                pid = gp.partition_id()
                for case in gp.Switch(pid, N_CORES):
                    gp.remote_dma_broadcast(
                        roots_nm[:, case, :], root16[:, :, 0],
                        remote_sem=recv_sem, local_sem=send_sem,
                        rdests=[(0, k) for k in range(N_CORES)],
                    ).then_inc(prep_sem, 1)
                gp.wait_ge(prep_sem, 1)
                gp.trigger_dma(1)
                cpy = nc.vector.tensor_copy(
                    nxt16, roots_nm.rearrange("p n t -> p t n"))
                cpy._wait_ge(recv_sem, 2 * N_CORES)
            cur = nxt16

            for P in top_rounds:
                cur = run_round(cur, P, True)

            # attention read over this core's batch slice
            psum_ctx.close()
            psa = ctx.enter_context(
                tc.tile_pool(name="psuma", bufs=1, space="PSUM"))
            att_ps = psa.tile([BPC, 7], f32, tag="att", name="att_ps")
            for t in range(G):
                nc.tensor.matmul(att_ps, ct_sb[:, t, :], mem_fm[:, t, :],
                                 start=(t == 0), stop=(t == G - 1))
            negmax = gate.tile([BPC, 1], f32, tag="negmax", name="negmax")
            nc.vector.tensor_reduce(negmax, att_ps, mybir.AxisListType.X,
                                    OP.max, negate=True)
            expv = gate.tile([BPC, 7], f32, tag="expv", name="expv")
            nc.scalar.activation(expv, att_ps, AF.Exp, bias=negmax)
            ssum = gate.tile([BPC, 1], f32, tag="ssum", name="ssum")
            nc.vector.tensor_reduce(ssum, expv, mybir.AxisListType.X, OP.add)
            rinv = gate.tile([BPC, 1], f32, tag="rinv", name="rinv")
            nc.vector.reciprocal(rinv, ssum)
            alpha = gate.tile([BPC, 7], f32, tag="alpha", name="alpha")
            nc.vector.tensor_scalar_mul(alpha, expv, rinv)

            memr = gate.tile([7, G, 128], f32, tag="memr", name="memr")
            for t in range(G):
                tps = psa.tile([7, 128], f32, tag="tps", name="tps")
                nc.tensor.transpose(tps, mem_fm[:, t, :], ident)
                nc.vector.tensor_copy(memr[:, t, :], tps)
            atp = psa.tile([7, BPC], f32, tag="atp", name="atp")
            nc.tensor.transpose(atp, alpha, ident[0:BPC, 0:BPC])
            alphat = gate.tile([7, BPC], f32, tag="alphat", name="alphat")
            nc.vector.tensor_copy(alphat, atp)

            out_sb = gate.tile([BPC, D], f32, tag="out_sb", name="out_sb")
            for half in range(2):
                ops = psa.tile([BPC, 512], f32, tag="ops", name="ops")
                nc.tensor.matmul(ops, alphat, memr[:, 4 * half:4 * (half + 1), :])
                nc.vector.tensor_copy(out_sb[:, 512 * half:512 * (half + 1)], ops)
            nc.sync.dma_start(out_d[:], out_sb[:])

    nc.compile()
    return nc


def _prep_inputs(queue, c_t, W, b):
    queue = np.asarray(queue, dtype=np.float32)
    c_t = np.asarray(c_t, dtype=np.float32)
    W = np.asarray(W, dtype=np.float32)
    b = np.asarray(b, dtype=np.float32)
    d = D
    WL = W[:, :d] + W[:, 2 * d:3 * d]
    WR = W[:, d:2 * d] + W[:, 3 * d:]
    wcat_t = np.concatenate([WL, WR], axis=1).T  # [2048, 3072]
    wt = np.ascontiguousarray(
        wcat_t.reshape(KT, 128, FT, 128).transpose(2, 1, 0, 3)).astype(np.float16)
    bb = np.ascontiguousarray(b.reshape(FT, 128)).astype(np.float32)
    in_maps = []
    for i in range(N_CORES):
        qs = queue[i * PER_CORE:(i + 1) * PER_CORE]
        qt = np.ascontiguousarray(
            qs.T.reshape(G, 128, PER_CORE)).astype(np.float16)
        ctt = np.ascontiguousarray(c_t[i * BPC:(i + 1) * BPC].T).astype(np.float32)
        in_maps.append({"wt": wt, "qt": qt, "ctt": ctt, "bb": bb})
    return in_maps


def kernel(queue, c_t, W, b):
    has_bias = bool(np.any(np.asarray(b)))
    nc = _CACHE.get(has_bias)
    if nc is None:
        nc = _build_nc(has_bias)
        _CACHE[has_bias] = nc
    in_maps = _prep_inputs(queue, c_t, W, b)
    from concourse.bass_utils import run_bass_kernel_spmd
    res = run_bass_kernel_spmd(nc, in_maps, core_ids=list(range(N_CORES)))
    return np.concatenate([r["out"] for r in res.results], axis=0)
